# revision 9
# baseline (speedup 1.0000x reference)
# Causal self-attention (GQA, RoPE) on 8 NeuronCores.
#
# Sharding: sequence-parallel. Core c = (batch b = c//4, role r = c%4).
# Each batch's 2048 tokens are split into 8 chunks of 256; role r owns
# chunks {r, 7-r} (zigzag, balances causal work). Each core computes
# QKV for its 512 tokens, AllGathers RoPE'd K^T and ones-augmented V
# within its 4-core batch group, runs causal attention for a uniform
# 24-slot schedule (SPMD needs identical instruction streams; per-core
# causal validity is data: invalid kv tiles contribute zero because
# their V tile incl. the ones column is zeroed), then projects its own
# token rows. No reduction needed after proj.
#
# All matmuls run in float32r (full PE speed at N>=256, ~1e-4 rel err).
# Softmax denominator is the 65th row of the AV matmul (ones column);
# no max subtraction (scores are bounded, fp32 exp is safe).
import sys

sys.path.insert(0, "/opt/trn_rl_repo")
import numpy as np

B, T, C = 2, 2048, 2048
NH, G, HS = 32, 8, 64
QPK = NH // G
NCORES = 8
CHUNK = 256
NCH = T // CHUNK          # 8 chunks per batch
NSLOT_B, NSLOT_A = 16, 8  # uniform kv 128-slots for the two q-chunks
NPREF = 14                # gathered prefix slots resident in SBUF

# Head ordering: q-tile t holds (EVEN_HEADS[t] at partitions 0-63,
# ODD_HEADS[t] at 64-127) so the K-slice partition base (g%2)*64 always
# matches the q-slice base.
EVEN_HEADS = [h for h in range(NH) if (h // QPK) % 2 == 0]
ODD_HEADS = [h for h in range(NH) if (h // QPK) % 2 == 1]


def tok_ids(r):
    a = list(range(r * CHUNK, (r + 1) * CHUNK))
    b = list(range((7 - r) * CHUNK, (8 - r) * CHUNK))
    return np.array(a + b, dtype=np.int64)


def perm_q():
    # reordered q feature j = t*128 + s*64 + d  ->  original attn_w row
    p = np.zeros(NH * HS, dtype=np.int64)
    for t in range(16):
        for s, h in ((0, EVEN_HEADS[t]), (1, ODD_HEADS[t])):
            g, qi = h // QPK, h % QPK
            for d in range(HS):
                p[t * 128 + s * 64 + d] = g * 384 + qi * 64 + d
    return p


def perm_k():
    p = np.zeros(G * HS, dtype=np.int64)
    for g in range(G):
        for d in range(HS):
            p[g * 64 + d] = g * 384 + 256 + d
    return p


def perm_v():
    p = np.zeros(G * HS, dtype=np.int64)
    for g in range(G):
        for d in range(HS):
            p[g * 64 + d] = g * 384 + 320 + d
    return p


def perm_y():
    # y^T row i = t*128 + s*64 + d -> proj_w column h*64+d
    p = np.zeros(NH * HS, dtype=np.int64)
    for t in range(16):
        for s, h in ((0, EVEN_HEADS[t]), (1, ODD_HEADS[t])):
            for d in range(HS):
                p[t * 128 + s * 64 + d] = h * 64 + d
    return p


def head_at(t, s):
    return EVEN_HEADS[t] if s == 0 else ODD_HEADS[t]


def slot_src(s):
    # gathered prefix slot s (kv 128-chunk index s) -> (rank, 256-chunk pos, col128)
    ci = s // 2
    if ci < 4:
        return ci, 0, s % 2
    return 7 - ci, 1, s % 2


def valid_tables(r):
    # validB[s]: qcB (chunk 7-r) prefix slot s valid; validA[s]: qcA (chunk r)
    vB = np.zeros(16, np.float32)
    vA = np.zeros(16, np.float32)
    for s in range(NPREF):
        vB[s] = 1.0 if s <= 13 - 2 * r else 0.0
    for s in range(6):
        vA[s] = 1.0 if s <= 2 * r - 1 else 0.0
    return vA, vB


def host_masks():
    i = np.arange(128)[:, None]
    j = np.arange(256)[None, :]
    m0 = (i <= j).astype(np.float32)
    m1 = (128 + i <= j).astype(np.float32)
    return m0, m1


_PROG = {}
MM_DTYPE = "bf16"   # "bf16" or "fp32r"


def _build_program():
    if "nc" in _PROG:
        return _PROG
    import concourse.bass as bass
    import concourse.tile as tile
    from concourse import bacc, mybir
    from contextlib import ExitStack

    f32 = mybir.dt.float32
    # fr = matmul operand dtype; fe = elementwise dtype feeding matmuls
    if MM_DTYPE == "bf16":
        fr = mybir.dt.bfloat16
        fe = mybir.dt.bfloat16
    else:
        fr = mybir.dt.float32r
        fe = mybir.dt.float32
    AF = mybir.ActivationFunctionType

    nc = bacc.Bacc("TRN2", target_bir_lowering=False, debug=False, num_devices=NCORES)

    xT_d = nc.dram_tensor("xT", [C, 512], fr, kind="ExternalInput").ap()
    wqkT_d = nc.dram_tensor("wqkT", [C, 2560], fr, kind="ExternalInput").ap()
    wvT_d = nc.dram_tensor("wvT", [C, 512], fr, kind="ExternalInput").ap()
    pwT_d = nc.dram_tensor("pwT", [C, C], fr, kind="ExternalInput").ap()
    bqk_d = nc.dram_tensor("bqk", [128, 20], f32, kind="ExternalInput").ap()
    bv_d = nc.dram_tensor("bv", [128, 512], f32, kind="ExternalInput").ap()
    pb_d = nc.dram_tensor("pb", [128, C], f32, kind="ExternalInput").ap()
    cos_d = nc.dram_tensor("cosT2", [128, 512], fe, kind="ExternalInput").ap()
    sin_d = nc.dram_tensor("sinT2s", [128, 512], fe, kind="ExternalInput").ap()
    mask_d = nc.dram_tensor("masks", [128, 2, 1024], fe, kind="ExternalInput").ap()
    valid_d = nc.dram_tensor("valid", [128, 32], f32, kind="ExternalInput").ap()
    vones_d = nc.dram_tensor("vones", [128, 4, 8], fr, kind="ExternalInput").ap()
    out_d = nc.dram_tensor("out", [512, C], f32, kind="ExternalOutput").ap()

    GROUPS = [[0, 1, 2, 3], [4, 5, 6, 7]]

    with tile.TileContext(nc) as tc:
        with ExitStack() as ctx:
            consts = ctx.enter_context(tc.tile_pool(name="consts", bufs=1))
            qy = ctx.enter_context(tc.tile_pool(name="qy", bufs=1))
            kvloc = ctx.enter_context(tc.tile_pool(name="kvloc", bufs=1))
            dram = ctx.enter_context(tc.tile_pool(name="dram", bufs=1, space="DRAM"))

            cos_s = consts.tile([128, 512], fe)
            sin_s = consts.tile([128, 512], fe)
            bqk_s = consts.tile([128, 20], f32)
            bv_s = consts.tile([128, 512], f32)
            pb_s = consts.tile([128, C], f32)
            mask_s = consts.tile([128, 2, 1024], fe)
            valid_s = consts.tile([128, 32], f32)
            nc.sync.dma_start(out=cos_s, in_=cos_d)
            nc.sync.dma_start(out=sin_s, in_=sin_d)
            nc.sync.dma_start(out=bqk_s, in_=bqk_d)
            nc.sync.dma_start(out=bv_s, in_=bv_d)
            nc.sync.dma_start(out=pb_s, in_=pb_d)
            nc.sync.dma_start(out=mask_s, in_=mask_d)
            nc.sync.dma_start(out=valid_s, in_=valid_d)

            qT_s = qy.tile([128, 16, 512], fr)
            yT_s = qy.tile([128, 16, 512], fr)
            kTl_s = kvloc.tile([128, 4, 512], fr)
            vAl_s = kvloc.tile([128, 4, 8, 65], fr)

            k_loc = dram.tile([4, 128, 512], fr)
            v_loc = dram.tile([4, 128, 8, 65], fr)
            k_gat = dram.tile([4, 4, 128, 512], fr)
            v_gat = dram.tile([4, 4, 128, 8, 65], fr)

            def rope_into(dst, ps, bias_col, rp):
                # dst[128, 512] (fp32r) <- rope(ps + bias); the 32-row
                # rotate-half swap is done by SBUF->SBUF DMAs (DVE needs
                # equal base partitions for two SBUF operands).
                tb = rp.tile([128, 512], fe, tag="tb", name="tb")
                nc.vector.tensor_scalar_add(tb, ps, bias_col)
                t2 = rp.tile([128, 512], fe, tag="t2", name="t2")
                nc.vector.tensor_mul(t2, tb, sin_s)
                tcs = rp.tile([128, 512], fe, tag="tc", name="tcs")
                nc.vector.tensor_mul(tcs, tb, cos_s)
                t2s = rp.tile([128, 512], fe, tag="t2s", name="t2s")
                for b0 in (0, 64):
                    nc.gpsimd.dma_start(out=t2s[b0:b0 + 32, :], in_=t2[b0 + 32:b0 + 64, :])
                    nc.gpsimd.dma_start(out=t2s[b0 + 32:b0 + 64, :], in_=t2[b0:b0 + 32, :])
                nc.vector.tensor_add(dst, t2s, tcs)

            # ---------------- phase 0: QKV projections ----------------
            with ExitStack() as p0:
                xp = p0.enter_context(tc.tile_pool(name="xp", bufs=1))
                rp = p0.enter_context(tc.tile_pool(name="rp", bufs=2))
                ps0 = p0.enter_context(tc.tile_pool(name="ps0", bufs=2, space="PSUM"))

                xT_s = xp.tile([128, 16, 512], fr)
                nc.sync.dma_start(out=xT_s, in_=xT_d.rearrange("(k p) t -> p k t", p=128))
                wq_all = xp.tile([128, 16, 2560], fr)
                for k4 in range(4):
                    nc.sync.dma_start(out=wq_all[:, k4 * 4:(k4 + 1) * 4, :],
                                      in_=wqkT_d[k4 * 512:(k4 + 1) * 512, :].rearrange("(j p) m -> p j m", p=128))
                wv_all = xp.tile([128, 16, 512], fr)
                nc.sync.dma_start(out=wv_all, in_=wvT_d.rearrange("(j p) m -> p j m", p=128))

                def qk_tile(dst, col0, bias_col):
                    ps = ps0.tile([128, 512], f32, tag="pk", name="ps")
                    for kc in range(16):
                        nc.tensor.matmul(ps, wq_all[:, kc, col0:col0 + 128], xT_s[:, kc, :],
                                         start=(kc == 0), stop=(kc == 15))
                    rope_into(dst, ps, bias_col, rp)

                # K^T tiles
                for kt in range(4):
                    qk_tile(kTl_s[:, kt, :], 2048 + kt * 128, bqk_s[:, 16 + kt:17 + kt])
                nc.sync.dma_start(out=k_loc.rearrange("k p t -> p k t"), in_=kTl_s)
                nc.gpsimd.collective_compute(
                    "AllGather", mybir.AluOpType.bypass, replica_groups=GROUPS,
                    ins=[k_loc.opt()], outs=[k_gat.opt()])

                # V tiles (natural layout, bias, ones column)
                ones_done = False
                psv = [ps0.tile([128, 512], f32, tag="pv", bufs=4, name=f"psv{mt}") for mt in range(4)]
                for kc in range(16):
                    for mt in range(4):
                        nc.tensor.matmul(psv[mt], xT_s[:, kc, mt * 128:(mt + 1) * 128], wv_all[:, kc, :],
                                         start=(kc == 0), stop=(kc == 15))
                nc.sync.dma_start(out=vAl_s[:, :, :, 64:65],
                                  in_=vones_d.rearrange("p c (g o) -> p c g o", o=1))
                for mt in range(4):
                    nc.vector.tensor_add(
                        vAl_s[:, mt, :, 0:64],
                        psv[mt].rearrange("p (g d) -> p g d", g=8),
                        bv_s.rearrange("p (g d) -> p g d", g=8))
                nc.sync.dma_start(out=v_loc.rearrange("c p g d -> p c g d"), in_=vAl_s)
                nc.gpsimd.collective_compute(
                    "AllGather", mybir.AluOpType.bypass, replica_groups=GROUPS,
                    ins=[v_loc.opt()], outs=[v_gat.opt()])

                # Q^T tiles
                for qt in range(16):
                    qk_tile(qT_s[:, qt, :], qt * 128, bqk_s[:, qt:qt + 1])

            # ---------------- phase 1: attention ----------------
            with ExitStack() as pa:
                at = pa.enter_context(tc.tile_pool(name="at", bufs=1))
                ep = pa.enter_context(tc.tile_pool(name="ep", bufs=6))
                psA = pa.enter_context(tc.tile_pool(name="psA", bufs=1, space="PSUM"))

                kg_s = at.tile([128, NPREF, 4, 128], fr)
                vg_s = at.tile([128, NPREF, 8, 65], fr)
                for s in range(NPREF):
                    rk, cp, half = slot_src(s)
                    nc.sync.dma_start(out=kg_s[:, s, :, :],
                                      in_=k_gat[rk, :, :, cp * 256 + half * 128: cp * 256 + (half + 1) * 128]
                                      .rearrange("k p t -> p k t"))
                    nc.sync.dma_start(out=vg_s[:, s, :, :], in_=v_gat[rk, cp * 2 + half, :, :, :])
                # stage qcB validity in place
                for s in range(NPREF):
                    nc.vector.tensor_scalar_mul(vg_s[:, s, :, :], vg_s[:, s, :, :].bitcast(fe), valid_s[:, s:s + 1])

                # Flat pipelined schedule over steps (qc, kt, slot, pair).
                # One step = one pair-position: 2 score matmuls (gi halves in
                # disjoint PE row groups), one gi-merged exp, 2 AV matmuls.
                # Lookahead-1 software pipeline crosses kt/qc boundaries so
                # neither the PE nor the scalar engine drains; avs PSUM
                # accumulators are copied to SBUF right at kt end so the next
                # kt's AV chain only waits on the copy, with the softmax
                # normalize chain running off the critical path.
                steps = []
                for qc, nslots in ((1, NSLOT_B), (0, NSLOT_A)):
                    for kt in range(4):
                        for slot in range(nslots):
                            for pair in range(2):
                                steps.append((qc, nslots, kt, slot, pair))

                avs_cur = {}

                def kv_slices(qc, kt, slot):
                    Ks, Vs = [], []
                    for gi in range(2):
                        g = 2 * kt + gi
                        if slot < 2:
                            Ks.append(kTl_s[gi * 64:(gi + 1) * 64, kt, qc * 256 + slot * 128: qc * 256 + (slot + 1) * 128])
                            Vs.append(vAl_s[:, qc * 2 + slot, g, :])
                        else:
                            Ks.append(kg_s[gi * 64:(gi + 1) * 64, slot - 2, kt, :])
                            Vs.append(vg_s[:, slot - 2, g, :])
                    return Ks, Vs

                def scores(step):
                    qc, nslots, kt, slot, pair = step
                    Ks, _ = kv_slices(qc, kt, slot)
                    pss = psA.tile([128, 1024], f32, tag="s", bufs=2, name="ps_s")
                    tA = kt * 4 + 2 * pair
                    for gi in range(2):
                        nc.tensor.matmul(pss[:, gi * 512:(gi + 1) * 512], Ks[gi],
                                         qT_s[gi * 64:(gi + 1) * 64, tA:tA + 2, qc * 256:(qc + 1) * 256],
                                         start=True, stop=True)
                    return pss

                def expstep(step, pss):
                    qc, nslots, kt, slot, pair = step
                    e = ep.tile([128, 1024], fr, tag="e", bufs=12, name="e")
                    nc.scalar.activation(e, pss, AF.Exp, scale=0.125)
                    if slot < 2:
                        # mask pattern repeats every 256 cols, so the same mask
                        # tile serves both gi halves
                        nc.vector.tensor_mul(e, e.bitcast(fe), mask_s[:, slot, :])
                    return e

                def av(step, e):
                    qc, nslots, kt, slot, pair = step
                    if slot == 0 and pair == 0:
                        avs_cur[(qc, kt)] = [[psA.tile([65, 512], f32, tag=f"av{gi}{j}", bufs=1,
                                                       name=f"ps_av{gi}{j}") for j in range(2)]
                                             for gi in range(2)]
                    avs = avs_cur[(qc, kt)]
                    _, Vs = kv_slices(qc, kt, slot)
                    for gi in range(2):
                        nc.tensor.matmul(avs[gi][pair], Vs[gi], e[:, gi * 512:(gi + 1) * 512],
                                         start=(slot == 0), stop=(slot == nslots - 1))

                def epilogue(qc, kt):
                    avs = avs_cur[(qc, kt)]
                    # One PSUM-freeing copy per accumulator, all emitted first:
                    # the next kt's AV chain only waits on these. The
                    # normalize chain then runs entirely off the SBUF copies.
                    yas = []
                    for gi in range(2):
                        for pair in range(2):
                            ya = ep.tile([65, 512], f32, tag="ya", bufs=4, name="ya")
                            nc.vector.tensor_copy(ya, avs[gi][pair])
                            yas.append(ya)
                    for gi in range(2):
                        for pair in range(2):
                            ya = yas[gi * 2 + pair]
                            rd = ep.tile([1, 512], f32, tag="rd", bufs=4, name="rd")
                            nc.vector.tensor_copy(rd, ya[64:65, :])
                            r_ = ep.tile([1, 512], f32, tag="r", bufs=4, name="r_")
                            nc.vector.reciprocal_approx_fast(r_, rd)
                            rb = ep.tile([64, 512], f32, tag="rb", bufs=4, name="rb")
                            nc.gpsimd.partition_broadcast(rb, r_)
                            tA = kt * 4 + 2 * pair
                            # normalize-mul on gpsimd: keeps the DVE queue free
                            # of the broadcast round-trip; only proj consumes yT
                            nc.gpsimd.tensor_mul(
                                yT_s[gi * 64:(gi + 1) * 64, tA:tA + 2, qc * 256:(qc + 1) * 256],
                                ya[0:64, :].rearrange("p (j t) -> p j t", j=2),
                                rb.rearrange("p (j t) -> p j t", j=2))

                es_cur = expstep(steps[0], scores(steps[0]))
                for i, st in enumerate(steps):
                    nxt = steps[i + 1] if i + 1 < len(steps) else None
                    pss_nxt = scores(nxt) if nxt is not None else None
                    av(st, es_cur)
                    if nxt is not None:
                        es_cur = expstep(nxt, pss_nxt)
                    qc, nslots, kt, slot, pair = st
                    if slot == nslots - 1 and pair == 1:
                        epilogue(qc, kt)
                        if qc == 1 and kt == 3:
                            # tighten validity for qcA (valid_A subset of valid_B)
                            for s in range(6):
                                nc.vector.tensor_scalar_mul(vg_s[:, s, :, :], vg_s[:, s, :, :].bitcast(fe),
                                                            valid_s[:, 16 + s:17 + s])

            # ---------------- phase 2: output projection ----------------
            with ExitStack() as pp:
                pr = pp.enter_context(tc.tile_pool(name="pr", bufs=4))
                psP = pp.enter_context(tc.tile_pool(name="psP", bufs=8, space="PSUM"))
                for n in range(4):
                    pss = [psP.tile([128, 512], f32, tag="pp", name=f"pss{mt}") for mt in range(4)]
                    for kd in range(16):
                        wpj = pr.tile([128, 512], fr, tag="wpj", name="wpj")
                        nc.scalar.dma_start(out=wpj, in_=pwT_d[kd * 128:(kd + 1) * 128, n * 512:(n + 1) * 512])
                        for mt in range(4):
                            nc.tensor.matmul(pss[mt], yT_s[:, kd, mt * 128:(mt + 1) * 128], wpj,
                                             start=(kd == 0), stop=(kd == 15))
                    for mt in range(4):
                        ost = pr.tile([128, 512], f32, tag="ost", name="ost")
                        nc.vector.tensor_add(ost, pss[mt], pb_s[:, n * 512:(n + 1) * 512])
                        nc.sync.dma_start(out=out_d[mt * 128:(mt + 1) * 128, n * 512:(n + 1) * 512], in_=ost)

    nc.compile()
    _PROG["nc"] = nc
    return _PROG


def make_in_maps(x, cos, sin, attn_w, attn_b, proj_w, proj_b):
    x = np.asarray(x, np.float32)
    cos = np.asarray(cos, np.float32)
    sin = np.asarray(sin, np.float32)
    attn_w = np.asarray(attn_w, np.float32)
    attn_b = np.asarray(attn_b, np.float32)
    proj_w = np.asarray(proj_w, np.float32)
    proj_b = np.asarray(proj_b, np.float32)

    pq, pk, pv, py = perm_q(), perm_k(), perm_v(), perm_y()
    pqk = np.concatenate([pq, pk])
    wqkT = np.ascontiguousarray(attn_w[pqk, :].T)          # [2048, 2560]
    wvT = np.ascontiguousarray(attn_w[pv, :].T)            # [2048, 512]
    pwT = np.ascontiguousarray(proj_w.T[py, :])            # [2048, 2048]
    bqk = np.ascontiguousarray(attn_b[pqk].reshape(20, 128).T)   # [128, 20]
    bv = np.tile(attn_b[pv][None, :], (128, 1)).astype(np.float32)
    pb = np.tile(proj_b[None, :], (128, 1)).astype(np.float32)
    m0, m1 = host_masks()
    masks = np.stack([m0, m1], axis=0).transpose(1, 0, 2)  # [128, 2, 256]
    masks = np.concatenate([masks] * 4, axis=2).copy()  # [128, 2, 1024] (four heads per exp tile)

    if MM_DTYPE == "bf16":
        import ml_dtypes
        mmt = ml_dtypes.bfloat16
    else:
        mmt = np.float32
    wqkT = wqkT.astype(mmt)
    wvT = wvT.astype(mmt)
    pwT = pwT.astype(mmt)
    masks = masks.astype(mmt)

    in_maps = []
    for c in range(NCORES):
        b, r = c // 4, c % 4
        ids = tok_ids(r)
        xT = np.ascontiguousarray(x[b, ids, :].T).astype(mmt)   # [2048, 512]
        cl = cos[ids, :].T                                  # [64, 512]
        sl = sin[ids, :].T.copy()
        sl[32:] *= -1.0
        cosT2 = np.concatenate([cl, cl], axis=0).astype(mmt)
        sinT2s = np.concatenate([sl, sl], axis=0).astype(mmt)
        vA, vB = valid_tables(r)
        valid = np.zeros((128, 32), np.float32)
        valid[:, 0:16] = vB[None, :]
        valid[:, 16:32] = vA[None, :]
        in_maps.append({
            "xT": xT, "wqkT": wqkT, "wvT": wvT, "pwT": pwT,
            "bqk": bqk, "bv": bv, "pb": pb,
            "cosT2": cosT2, "sinT2s": sinT2s, "masks": masks, "valid": valid,
            "vones": np.ones((128, 4, 8), mmt),
        })
    return in_maps


def assemble_output(results):
    out = np.zeros((B, T, C), np.float32)
    for c in range(NCORES):
        b, r = c // 4, c % 4
        ids = tok_ids(r)
        out[b, ids, :] = results[c]["out"]
    return out


def kernel(**inputs):
    from concourse.bass_utils import run_bass_kernel_spmd

    prog = _build_program()
    in_maps = make_in_maps(**inputs)
    res = run_bass_kernel_spmd(prog["nc"], in_maps, list(range(NCORES)))
    return assemble_output(res.results)


if __name__ == "__main__":
    import reference

    inputs = {k: np.asarray(v) for k, v in reference.setup_inputs().items()}
    expected = np.asarray(reference.reference(**inputs))
    actual = kernel(**inputs)
    err = np.abs(actual - expected).max()
    rel = np.abs(actual - expected).max() / np.abs(expected).max()
    print(f"abs={err:.3e} rel={rel:.3e}")



# revision 10
# speedup vs baseline: 1.7375x; 1.7375x over previous
# Causal self-attention (GQA, RoPE) on 8 NeuronCores.
#
# Sharding: sequence-parallel. Core c = (batch b = c//4, role r = c%4).
# Each batch's 2048 tokens are split into 8 chunks of 256; role r owns
# chunks {r, 7-r} (zigzag, balances causal work). Each core computes
# QKV for its 512 tokens, AllGathers RoPE'd K^T and ones-augmented V
# within its 4-core batch group, runs causal attention for a uniform
# 24-slot schedule (SPMD needs identical instruction streams; per-core
# causal validity is data: invalid kv tiles contribute zero because
# their V tile incl. the ones column is zeroed), then projects its own
# token rows. No reduction needed after proj.
#
# All matmuls run in float32r (full PE speed at N>=256, ~1e-4 rel err).
# Softmax denominator is the 65th row of the AV matmul (ones column);
# no max subtraction (scores are bounded, fp32 exp is safe).
import sys

sys.path.insert(0, "/opt/trn_rl_repo")
import numpy as np

B, T, C = 2, 2048, 2048
NH, G, HS = 32, 8, 64
QPK = NH // G
NCORES = 8
CHUNK = 256
NCH = T // CHUNK          # 8 chunks per batch
NSLOT_B, NSLOT_A = 16, 8  # uniform kv 128-slots for the two q-chunks
NPREF = 14                # gathered prefix slots resident in SBUF

# Head ordering: q-tile t holds (EVEN_HEADS[t] at partitions 0-63,
# ODD_HEADS[t] at 64-127) so the K-slice partition base (g%2)*64 always
# matches the q-slice base.
EVEN_HEADS = [h for h in range(NH) if (h // QPK) % 2 == 0]
ODD_HEADS = [h for h in range(NH) if (h // QPK) % 2 == 1]


def tok_ids(r):
    a = list(range(r * CHUNK, (r + 1) * CHUNK))
    b = list(range((7 - r) * CHUNK, (8 - r) * CHUNK))
    return np.array(a + b, dtype=np.int64)


def perm_q():
    # reordered q feature j = t*128 + s*64 + d  ->  original attn_w row
    p = np.zeros(NH * HS, dtype=np.int64)
    for t in range(16):
        for s, h in ((0, EVEN_HEADS[t]), (1, ODD_HEADS[t])):
            g, qi = h // QPK, h % QPK
            for d in range(HS):
                p[t * 128 + s * 64 + d] = g * 384 + qi * 64 + d
    return p


def perm_k():
    p = np.zeros(G * HS, dtype=np.int64)
    for g in range(G):
        for d in range(HS):
            p[g * 64 + d] = g * 384 + 256 + d
    return p


def perm_v():
    p = np.zeros(G * HS, dtype=np.int64)
    for g in range(G):
        for d in range(HS):
            p[g * 64 + d] = g * 384 + 320 + d
    return p


def perm_y():
    # y^T row i = t*128 + s*64 + d -> proj_w column h*64+d
    p = np.zeros(NH * HS, dtype=np.int64)
    for t in range(16):
        for s, h in ((0, EVEN_HEADS[t]), (1, ODD_HEADS[t])):
            for d in range(HS):
                p[t * 128 + s * 64 + d] = h * 64 + d
    return p


def head_at(t, s):
    return EVEN_HEADS[t] if s == 0 else ODD_HEADS[t]


def slot_src(s):
    # gathered prefix slot s (kv 128-chunk index s) -> (rank, 256-chunk pos, col128)
    ci = s // 2
    if ci < 4:
        return ci, 0, s % 2
    return 7 - ci, 1, s % 2


def valid_tables(r):
    # validB[s]: qcB (chunk 7-r) prefix slot s valid; validA[s]: qcA (chunk r)
    vB = np.zeros(16, np.float32)
    vA = np.zeros(16, np.float32)
    for s in range(NPREF):
        vB[s] = 1.0 if s <= 13 - 2 * r else 0.0
    for s in range(6):
        vA[s] = 1.0 if s <= 2 * r - 1 else 0.0
    return vA, vB


def host_masks():
    i = np.arange(128)[:, None]
    j = np.arange(256)[None, :]
    m0 = (i <= j).astype(np.float32)
    m1 = (128 + i <= j).astype(np.float32)
    return m0, m1


_PROG = {}
MM_DTYPE = "bf16"   # "bf16" or "fp32r"


def _build_program():
    if "nc" in _PROG:
        return _PROG
    import concourse.bass as bass
    import concourse.tile as tile
    from concourse import bacc, mybir
    from contextlib import ExitStack

    f32 = mybir.dt.float32
    # fr = matmul operand dtype; fe = elementwise dtype feeding matmuls
    if MM_DTYPE == "bf16":
        fr = mybir.dt.bfloat16
        fe = mybir.dt.bfloat16
    else:
        fr = mybir.dt.float32r
        fe = mybir.dt.float32
    AF = mybir.ActivationFunctionType

    nc = bacc.Bacc("TRN2", target_bir_lowering=False, debug=False, num_devices=NCORES)

    xT_d = nc.dram_tensor("xT", [C, 512], fr, kind="ExternalInput").ap()
    wqkT_d = nc.dram_tensor("wqkT", [C, 2560], fr, kind="ExternalInput").ap()
    wvT_d = nc.dram_tensor("wvT", [C, 512], fr, kind="ExternalInput").ap()
    pwT_d = nc.dram_tensor("pwT", [C, C], fr, kind="ExternalInput").ap()
    bqk_d = nc.dram_tensor("bqk", [128, 20], f32, kind="ExternalInput").ap()
    bv_d = nc.dram_tensor("bv", [128, 512], f32, kind="ExternalInput").ap()
    pb_d = nc.dram_tensor("pb", [128, C], f32, kind="ExternalInput").ap()
    cos_d = nc.dram_tensor("cosT2", [128, 512], fe, kind="ExternalInput").ap()
    sin_d = nc.dram_tensor("sinT2s", [128, 512], fe, kind="ExternalInput").ap()
    mask_d = nc.dram_tensor("masks", [128, 2, 1024], fe, kind="ExternalInput").ap()
    valid_d = nc.dram_tensor("valid", [128, 32], f32, kind="ExternalInput").ap()
    vones_d = nc.dram_tensor("vones", [128, 4, 8], fr, kind="ExternalInput").ap()
    out_d = nc.dram_tensor("out", [512, C], f32, kind="ExternalOutput").ap()

    GROUPS = [[0, 1, 2, 3], [4, 5, 6, 7]]

    with tile.TileContext(nc) as tc:
        with ExitStack() as ctx:
            consts = ctx.enter_context(tc.tile_pool(name="consts", bufs=1))
            qy = ctx.enter_context(tc.tile_pool(name="qy", bufs=1))
            kvloc = ctx.enter_context(tc.tile_pool(name="kvloc", bufs=1))
            dram = ctx.enter_context(tc.tile_pool(name="dram", bufs=1, space="DRAM"))

            cos_s = consts.tile([128, 512], fe)
            sin_s = consts.tile([128, 512], fe)
            bqk_s = consts.tile([128, 20], f32)
            bv_s = consts.tile([128, 512], f32)
            pb_s = consts.tile([128, C], f32)
            mask_s = consts.tile([128, 2, 1024], fe)
            valid_s = consts.tile([128, 32], f32)
            nc.sync.dma_start(out=cos_s, in_=cos_d)
            nc.sync.dma_start(out=sin_s, in_=sin_d)
            nc.sync.dma_start(out=bqk_s, in_=bqk_d)
            nc.sync.dma_start(out=bv_s, in_=bv_d)
            nc.sync.dma_start(out=pb_s, in_=pb_d)
            nc.sync.dma_start(out=mask_s, in_=mask_d)
            nc.sync.dma_start(out=valid_s, in_=valid_d)

            qT_s = qy.tile([128, 16, 512], fr)
            yT_s = qy.tile([128, 16, 512], fr)
            kTl_s = kvloc.tile([128, 4, 512], fr)
            vAl_s = kvloc.tile([128, 4, 8, 65], fr)

            k_loc = dram.tile([4, 128, 512], fr)
            v_loc = dram.tile([4, 128, 8, 65], fr)
            k_gat = dram.tile([4, 4, 128, 512], fr)
            v_gat = dram.tile([4, 4, 128, 8, 65], fr)

            def rope_into(dst, ps, bias_col, rp):
                # dst[128, 512] (fp32r) <- rope(ps + bias); the 32-row
                # rotate-half swap is done by SBUF->SBUF DMAs (DVE needs
                # equal base partitions for two SBUF operands).
                tb = rp.tile([128, 512], fe, tag="tb", name="tb")
                nc.vector.tensor_scalar_add(tb, ps, bias_col)
                t2 = rp.tile([128, 512], fe, tag="t2", name="t2")
                nc.vector.tensor_mul(t2, tb, sin_s)
                tcs = rp.tile([128, 512], fe, tag="tc", name="tcs")
                nc.vector.tensor_mul(tcs, tb, cos_s)
                t2s = rp.tile([128, 512], fe, tag="t2s", name="t2s")
                for b0 in (0, 64):
                    nc.gpsimd.dma_start(out=t2s[b0:b0 + 32, :], in_=t2[b0 + 32:b0 + 64, :])
                    nc.gpsimd.dma_start(out=t2s[b0 + 32:b0 + 64, :], in_=t2[b0:b0 + 32, :])
                nc.vector.tensor_add(dst, t2s, tcs)

            # ---------------- phase 0: QKV projections ----------------
            with ExitStack() as p0:
                xp = p0.enter_context(tc.tile_pool(name="xp", bufs=1))
                rp = p0.enter_context(tc.tile_pool(name="rp", bufs=2))
                ps0 = p0.enter_context(tc.tile_pool(name="ps0", bufs=2, space="PSUM"))

                xT_s = xp.tile([128, 16, 512], fr)
                nc.sync.dma_start(out=xT_s, in_=xT_d.rearrange("(k p) t -> p k t", p=128))
                wq_all = xp.tile([128, 16, 2560], fr)
                for k4 in range(4):
                    nc.sync.dma_start(out=wq_all[:, k4 * 4:(k4 + 1) * 4, :],
                                      in_=wqkT_d[k4 * 512:(k4 + 1) * 512, :].rearrange("(j p) m -> p j m", p=128))
                wv_all = xp.tile([128, 16, 512], fr)
                nc.sync.dma_start(out=wv_all, in_=wvT_d.rearrange("(j p) m -> p j m", p=128))

                def qk_tile(dst, col0, bias_col):
                    ps = ps0.tile([128, 512], f32, tag="pk", name="ps")
                    for kc in range(16):
                        nc.tensor.matmul(ps, wq_all[:, kc, col0:col0 + 128], xT_s[:, kc, :],
                                         start=(kc == 0), stop=(kc == 15))
                    rope_into(dst, ps, bias_col, rp)

                # K^T tiles
                for kt in range(4):
                    qk_tile(kTl_s[:, kt, :], 2048 + kt * 128, bqk_s[:, 16 + kt:17 + kt])
                nc.sync.dma_start(out=k_loc.rearrange("k p t -> p k t"), in_=kTl_s)
                nc.gpsimd.collective_compute(
                    "AllGather", mybir.AluOpType.bypass, replica_groups=GROUPS,
                    ins=[k_loc.opt()], outs=[k_gat.opt()])

                # V tiles (natural layout, bias, ones column)
                ones_done = False
                psv = [ps0.tile([128, 512], f32, tag="pv", bufs=4, name=f"psv{mt}") for mt in range(4)]
                for kc in range(16):
                    for mt in range(4):
                        nc.tensor.matmul(psv[mt], xT_s[:, kc, mt * 128:(mt + 1) * 128], wv_all[:, kc, :],
                                         start=(kc == 0), stop=(kc == 15))
                nc.sync.dma_start(out=vAl_s[:, :, :, 64:65],
                                  in_=vones_d.rearrange("p c (g o) -> p c g o", o=1))
                for mt in range(4):
                    nc.vector.tensor_add(
                        vAl_s[:, mt, :, 0:64],
                        psv[mt].rearrange("p (g d) -> p g d", g=8),
                        bv_s.rearrange("p (g d) -> p g d", g=8))
                nc.sync.dma_start(out=v_loc.rearrange("c p g d -> p c g d"), in_=vAl_s)
                nc.gpsimd.collective_compute(
                    "AllGather", mybir.AluOpType.bypass, replica_groups=GROUPS,
                    ins=[v_loc.opt()], outs=[v_gat.opt()])

                # Q^T tiles
                for qt in range(16):
                    qk_tile(qT_s[:, qt, :], qt * 128, bqk_s[:, qt:qt + 1])

            # ---------------- phase 1: attention ----------------
            with ExitStack() as pa:
                at = pa.enter_context(tc.tile_pool(name="at", bufs=1))
                ep = pa.enter_context(tc.tile_pool(name="ep", bufs=6))
                psA = pa.enter_context(tc.tile_pool(name="psA", bufs=1, space="PSUM"))

                kg_s = at.tile([128, NPREF, 4, 128], fr)
                vg_s = at.tile([128, NPREF, 8, 65], fr)
                for s in range(NPREF):
                    rk, cp, half = slot_src(s)
                    nc.sync.dma_start(out=kg_s[:, s, :, :],
                                      in_=k_gat[rk, :, :, cp * 256 + half * 128: cp * 256 + (half + 1) * 128]
                                      .rearrange("k p t -> p k t"))
                    nc.sync.dma_start(out=vg_s[:, s, :, :], in_=v_gat[rk, cp * 2 + half, :, :, :])
                # stage qcB validity in place
                for s in range(NPREF):
                    nc.vector.tensor_scalar_mul(vg_s[:, s, :, :], vg_s[:, s, :, :].bitcast(fe), valid_s[:, s:s + 1])

                # Flat pipelined schedule over steps (qc, kt, slot, pair).
                # One step = one pair-position: 2 score matmuls (gi halves in
                # disjoint PE row groups), one gi-merged exp, 2 AV matmuls.
                # Lookahead-1 software pipeline crosses kt/qc boundaries so
                # neither the PE nor the scalar engine drains; avs PSUM
                # accumulators are copied to SBUF right at kt end so the next
                # kt's AV chain only waits on the copy, with the softmax
                # normalize chain running off the critical path.
                steps = []
                for qc, nslots in ((1, NSLOT_B), (0, NSLOT_A)):
                    for kt in range(4):
                        for slot in range(nslots):
                            for pair in range(2):
                                steps.append((qc, nslots, kt, slot, pair))

                avs_cur = {}

                def kv_slices(qc, kt, slot):
                    Ks, Vs = [], []
                    for gi in range(2):
                        g = 2 * kt + gi
                        if slot < 2:
                            Ks.append(kTl_s[gi * 64:(gi + 1) * 64, kt, qc * 256 + slot * 128: qc * 256 + (slot + 1) * 128])
                            Vs.append(vAl_s[:, qc * 2 + slot, g, :])
                        else:
                            Ks.append(kg_s[gi * 64:(gi + 1) * 64, slot - 2, kt, :])
                            Vs.append(vg_s[:, slot - 2, g, :])
                    return Ks, Vs

                def scores(step):
                    qc, nslots, kt, slot, pair = step
                    Ks, _ = kv_slices(qc, kt, slot)
                    pss = psA.tile([128, 1024], f32, tag="s", bufs=2, name="ps_s")
                    tA = kt * 4 + 2 * pair
                    for gi in range(2):
                        nc.tensor.matmul(pss[:, gi * 512:(gi + 1) * 512], Ks[gi],
                                         qT_s[gi * 64:(gi + 1) * 64, tA:tA + 2, qc * 256:(qc + 1) * 256],
                                         start=True, stop=True)
                    return pss

                def expstep(step, pss):
                    qc, nslots, kt, slot, pair = step
                    e = ep.tile([128, 1024], fr, tag="e", bufs=12, name="e")
                    nc.scalar.activation(e, pss, AF.Exp, scale=0.125)
                    if slot < 2:
                        # mask pattern repeats every 256 cols, so the same mask
                        # tile serves both gi halves
                        nc.vector.tensor_mul(e, e.bitcast(fe), mask_s[:, slot, :])
                    return e

                def av(step, e):
                    qc, nslots, kt, slot, pair = step
                    if slot == 0 and pair == 0:
                        avs_cur[(qc, kt)] = [[psA.tile([65, 512], f32, tag=f"av{gi}{j}", bufs=1,
                                                       name=f"ps_av{gi}{j}") for j in range(2)]
                                             for gi in range(2)]
                    avs = avs_cur[(qc, kt)]
                    _, Vs = kv_slices(qc, kt, slot)
                    for gi in range(2):
                        nc.tensor.matmul(avs[gi][pair], Vs[gi], e[:, gi * 512:(gi + 1) * 512],
                                         start=(slot == 0), stop=(slot == nslots - 1))

                def epilogue(qc, kt):
                    avs = avs_cur[(qc, kt)]
                    # One PSUM-freeing copy per accumulator, all emitted first:
                    # the next kt's AV chain only waits on these. The
                    # normalize chain then runs entirely off the SBUF copies.
                    yas = []
                    for gi in range(2):
                        for pair in range(2):
                            ya = ep.tile([65, 512], f32, tag="ya", bufs=4, name="ya")
                            nc.vector.tensor_copy(ya, avs[gi][pair])
                            yas.append(ya)
                    for gi in range(2):
                        for pair in range(2):
                            ya = yas[gi * 2 + pair]
                            rd = ep.tile([1, 512], f32, tag="rd", bufs=4, name="rd")
                            nc.vector.tensor_copy(rd, ya[64:65, :])
                            r_ = ep.tile([1, 512], f32, tag="r", bufs=4, name="r_")
                            nc.vector.reciprocal_approx_fast(r_, rd)
                            rb = ep.tile([64, 512], f32, tag="rb", bufs=4, name="rb")
                            nc.gpsimd.partition_broadcast(rb, r_)
                            tA = kt * 4 + 2 * pair
                            nc.vector.tensor_mul(
                                yT_s[gi * 64:(gi + 1) * 64, tA:tA + 2, qc * 256:(qc + 1) * 256],
                                ya[0:64, :].rearrange("p (j t) -> p j t", j=2),
                                rb.rearrange("p (j t) -> p j t", j=2))

                es_cur = expstep(steps[0], scores(steps[0]))
                for i, st in enumerate(steps):
                    nxt = steps[i + 1] if i + 1 < len(steps) else None
                    pss_nxt = scores(nxt) if nxt is not None else None
                    av(st, es_cur)
                    if nxt is not None:
                        es_cur = expstep(nxt, pss_nxt)
                    qc, nslots, kt, slot, pair = st
                    if slot == nslots - 1 and pair == 1:
                        epilogue(qc, kt)
                        if qc == 1 and kt == 3:
                            # tighten validity for qcA (valid_A subset of valid_B)
                            for s in range(6):
                                nc.vector.tensor_scalar_mul(vg_s[:, s, :, :], vg_s[:, s, :, :].bitcast(fe),
                                                            valid_s[:, 16 + s:17 + s])

            # ---------------- phase 2: output projection ----------------
            with ExitStack() as pp:
                pr = pp.enter_context(tc.tile_pool(name="pr", bufs=4))
                psP = pp.enter_context(tc.tile_pool(name="psP", bufs=8, space="PSUM"))
                for n in range(4):
                    pss = [psP.tile([128, 512], f32, tag="pp", name=f"pss{mt}") for mt in range(4)]
                    for kd in range(16):
                        wpj = pr.tile([128, 512], fr, tag="wpj", name="wpj")
                        nc.scalar.dma_start(out=wpj, in_=pwT_d[kd * 128:(kd + 1) * 128, n * 512:(n + 1) * 512])
                        for mt in range(4):
                            nc.tensor.matmul(pss[mt], yT_s[:, kd, mt * 128:(mt + 1) * 128], wpj,
                                             start=(kd == 0), stop=(kd == 15))
                    for mt in range(4):
                        ost = pr.tile([128, 512], f32, tag="ost", name="ost")
                        nc.vector.tensor_add(ost, pss[mt], pb_s[:, n * 512:(n + 1) * 512])
                        nc.sync.dma_start(out=out_d[mt * 128:(mt + 1) * 128, n * 512:(n + 1) * 512], in_=ost)

    nc.compile()
    _PROG["nc"] = nc
    return _PROG


def make_in_maps(x, cos, sin, attn_w, attn_b, proj_w, proj_b):
    x = np.asarray(x, np.float32)
    cos = np.asarray(cos, np.float32)
    sin = np.asarray(sin, np.float32)
    attn_w = np.asarray(attn_w, np.float32)
    attn_b = np.asarray(attn_b, np.float32)
    proj_w = np.asarray(proj_w, np.float32)
    proj_b = np.asarray(proj_b, np.float32)

    pq, pk, pv, py = perm_q(), perm_k(), perm_v(), perm_y()
    pqk = np.concatenate([pq, pk])
    wqkT = np.ascontiguousarray(attn_w[pqk, :].T)          # [2048, 2560]
    wvT = np.ascontiguousarray(attn_w[pv, :].T)            # [2048, 512]
    pwT = np.ascontiguousarray(proj_w.T[py, :])            # [2048, 2048]
    bqk = np.ascontiguousarray(attn_b[pqk].reshape(20, 128).T)   # [128, 20]
    bv = np.tile(attn_b[pv][None, :], (128, 1)).astype(np.float32)
    pb = np.tile(proj_b[None, :], (128, 1)).astype(np.float32)
    m0, m1 = host_masks()
    masks = np.stack([m0, m1], axis=0).transpose(1, 0, 2)  # [128, 2, 256]
    masks = np.concatenate([masks] * 4, axis=2).copy()  # [128, 2, 1024] (four heads per exp tile)

    if MM_DTYPE == "bf16":
        import ml_dtypes
        mmt = ml_dtypes.bfloat16
    else:
        mmt = np.float32
    wqkT = wqkT.astype(mmt)
    wvT = wvT.astype(mmt)
    pwT = pwT.astype(mmt)
    masks = masks.astype(mmt)

    in_maps = []
    for c in range(NCORES):
        b, r = c // 4, c % 4
        ids = tok_ids(r)
        xT = np.ascontiguousarray(x[b, ids, :].T).astype(mmt)   # [2048, 512]
        cl = cos[ids, :].T                                  # [64, 512]
        sl = sin[ids, :].T.copy()
        sl[32:] *= -1.0
        cosT2 = np.concatenate([cl, cl], axis=0).astype(mmt)
        sinT2s = np.concatenate([sl, sl], axis=0).astype(mmt)
        vA, vB = valid_tables(r)
        valid = np.zeros((128, 32), np.float32)
        valid[:, 0:16] = vB[None, :]
        valid[:, 16:32] = vA[None, :]
        in_maps.append({
            "xT": xT, "wqkT": wqkT, "wvT": wvT, "pwT": pwT,
            "bqk": bqk, "bv": bv, "pb": pb,
            "cosT2": cosT2, "sinT2s": sinT2s, "masks": masks, "valid": valid,
            "vones": np.ones((128, 4, 8), mmt),
        })
    return in_maps


def assemble_output(results):
    out = np.zeros((B, T, C), np.float32)
    for c in range(NCORES):
        b, r = c // 4, c % 4
        ids = tok_ids(r)
        out[b, ids, :] = results[c]["out"]
    return out


def kernel(**inputs):
    from concourse.bass_utils import run_bass_kernel_spmd

    prog = _build_program()
    in_maps = make_in_maps(**inputs)
    res = run_bass_kernel_spmd(prog["nc"], in_maps, list(range(NCORES)))
    return assemble_output(res.results)


if __name__ == "__main__":
    import reference

    inputs = {k: np.asarray(v) for k, v in reference.setup_inputs().items()}
    expected = np.asarray(reference.reference(**inputs))
    actual = kernel(**inputs)
    err = np.abs(actual - expected).max()
    rel = np.abs(actual - expected).max() / np.abs(expected).max()
    print(f"abs={err:.3e} rel={rel:.3e}")



# revision 16
# speedup vs baseline: 1.9348x; 1.1136x over previous
# Causal self-attention (GQA, RoPE) on 8 NeuronCores.
#
# Sharding: sequence-parallel. Core c = (batch b = c//4, role r = c%4).
# Each batch's 2048 tokens are split into 8 chunks of 256; role r owns
# chunks {r, 7-r} (zigzag, balances causal work). Each core computes
# QKV for its 512 tokens, AllGathers RoPE'd K^T and ones-augmented V
# within its 4-core batch group, runs causal attention for a uniform
# 24-slot schedule (SPMD needs identical instruction streams; per-core
# causal validity is data: invalid kv tiles contribute zero because
# their V tile incl. the ones column is zeroed), then projects its own
# token rows. No reduction needed after proj.
#
# All matmuls run in float32r (full PE speed at N>=256, ~1e-4 rel err).
# Softmax denominator is the 65th row of the AV matmul (ones column);
# no max subtraction (scores are bounded, fp32 exp is safe).
import sys

sys.path.insert(0, "/opt/trn_rl_repo")
import numpy as np

B, T, C = 2, 2048, 2048
NH, G, HS = 32, 8, 64
QPK = NH // G
NCORES = 8
CHUNK = 256
NCH = T // CHUNK          # 8 chunks per batch
NSLOT_B, NSLOT_A = 16, 8  # uniform kv 128-slots for the two q-chunks
NPREF = 14                # gathered prefix slots resident in SBUF

# Head ordering: q-tile t holds (EVEN_HEADS[t] at partitions 0-63,
# ODD_HEADS[t] at 64-127) so the K-slice partition base (g%2)*64 always
# matches the q-slice base.
EVEN_HEADS = [h for h in range(NH) if (h // QPK) % 2 == 0]
ODD_HEADS = [h for h in range(NH) if (h // QPK) % 2 == 1]


def tok_ids(r):
    a = list(range(r * CHUNK, (r + 1) * CHUNK))
    b = list(range((7 - r) * CHUNK, (8 - r) * CHUNK))
    return np.array(a + b, dtype=np.int64)


def perm_q():
    # reordered q feature j = t*128 + s*64 + d  ->  original attn_w row
    p = np.zeros(NH * HS, dtype=np.int64)
    for t in range(16):
        for s, h in ((0, EVEN_HEADS[t]), (1, ODD_HEADS[t])):
            g, qi = h // QPK, h % QPK
            for d in range(HS):
                p[t * 128 + s * 64 + d] = g * 384 + qi * 64 + d
    return p


def perm_k():
    p = np.zeros(G * HS, dtype=np.int64)
    for g in range(G):
        for d in range(HS):
            p[g * 64 + d] = g * 384 + 256 + d
    return p


def perm_v():
    p = np.zeros(G * HS, dtype=np.int64)
    for g in range(G):
        for d in range(HS):
            p[g * 64 + d] = g * 384 + 320 + d
    return p


def perm_y():
    # y^T row i = t*128 + s*64 + d -> proj_w column h*64+d
    p = np.zeros(NH * HS, dtype=np.int64)
    for t in range(16):
        for s, h in ((0, EVEN_HEADS[t]), (1, ODD_HEADS[t])):
            for d in range(HS):
                p[t * 128 + s * 64 + d] = h * 64 + d
    return p


def head_at(t, s):
    return EVEN_HEADS[t] if s == 0 else ODD_HEADS[t]


def slot_src(s):
    # gathered prefix slot s (kv 128-chunk index s) -> (rank, 256-chunk pos, col128)
    ci = s // 2
    if ci < 4:
        return ci, 0, s % 2
    return 7 - ci, 1, s % 2


def valid_tables(r):
    # validB[s]: qcB (chunk 7-r) prefix slot s valid; validA[s]: qcA (chunk r)
    vB = np.zeros(16, np.float32)
    vA = np.zeros(16, np.float32)
    for s in range(NPREF):
        vB[s] = 1.0 if s <= 13 - 2 * r else 0.0
    for s in range(6):
        vA[s] = 1.0 if s <= 2 * r - 1 else 0.0
    return vA, vB


def host_masks():
    # additive causal masks for the two diagonal kv slots: 0 where valid,
    # -240 on future positions (exp(0.125*(s-240)) ~ 1e-11)
    i = np.arange(128)[:, None]
    j = np.arange(256)[None, :]
    m0 = np.where(i <= j, 0.0, -240.0).astype(np.float32)
    m1 = np.where(128 + i <= j, 0.0, -240.0).astype(np.float32)
    return m0, m1


_PROG = {}
MM_DTYPE = "bf16"   # "bf16" or "fp32r"


def _build_program():
    if "nc" in _PROG:
        return _PROG
    import concourse.bass as bass
    import concourse.tile as tile
    from concourse import bacc, mybir
    from contextlib import ExitStack

    f32 = mybir.dt.float32
    # fr = matmul operand dtype; fe = elementwise dtype feeding matmuls
    if MM_DTYPE == "bf16":
        fr = mybir.dt.bfloat16
        fe = mybir.dt.bfloat16
    else:
        fr = mybir.dt.float32r
        fe = mybir.dt.float32
    AF = mybir.ActivationFunctionType

    nc = bacc.Bacc("TRN2", target_bir_lowering=False, debug=False, num_devices=NCORES)

    xT_d = nc.dram_tensor("xT", [C, 512], fr, kind="ExternalInput").ap()
    wqkT_d = nc.dram_tensor("wqkT", [C, 2560], fr, kind="ExternalInput").ap()
    wvT_d = nc.dram_tensor("wvT", [C, 512], fr, kind="ExternalInput").ap()
    pwT_d = nc.dram_tensor("pwT", [C, C], fr, kind="ExternalInput").ap()
    bqk_d = nc.dram_tensor("bqk", [128, 20], f32, kind="ExternalInput").ap()
    bv_d = nc.dram_tensor("bv", [128, 512], f32, kind="ExternalInput").ap()
    pb_d = nc.dram_tensor("pb", [128, C], f32, kind="ExternalInput").ap()
    cos_d = nc.dram_tensor("cosT2", [128, 512], fe, kind="ExternalInput").ap()
    sin_d = nc.dram_tensor("sinT2s", [128, 512], fe, kind="ExternalInput").ap()
    ident_d = nc.dram_tensor("ident", [128, 128], fr, kind="ExternalInput").ap()
    rmask_d = nc.dram_tensor("rmask", [128, 2, 512], fr, kind="ExternalInput").ap()
    valid_d = nc.dram_tensor("valid", [128, 32], f32, kind="ExternalInput").ap()
    vones_d = nc.dram_tensor("vones", [128, 4, 8], fr, kind="ExternalInput").ap()
    out_d = nc.dram_tensor("out", [512, C], f32, kind="ExternalOutput").ap()

    GROUPS = [[0, 1, 2, 3], [4, 5, 6, 7]]

    with tile.TileContext(nc) as tc:
        with ExitStack() as ctx:
            consts = ctx.enter_context(tc.tile_pool(name="consts", bufs=1))
            qy = ctx.enter_context(tc.tile_pool(name="qy", bufs=1))
            kvloc = ctx.enter_context(tc.tile_pool(name="kvloc", bufs=1))
            dram = ctx.enter_context(tc.tile_pool(name="dram", bufs=1, space="DRAM"))

            cos_s = consts.tile([128, 512], fe)
            sin_s = consts.tile([128, 512], fe)
            bqk_s = consts.tile([128, 20], f32)
            bv_s = consts.tile([128, 512], f32)
            pb_s = consts.tile([128, C], f32)
            ident_s = consts.tile([128, 128], fr)
            rmask_s = consts.tile([128, 2, 512], fr)
            valid_s = consts.tile([128, 32], f32)
            nc.sync.dma_start(out=cos_s, in_=cos_d)
            nc.sync.dma_start(out=sin_s, in_=sin_d)
            nc.sync.dma_start(out=bqk_s, in_=bqk_d)
            nc.sync.dma_start(out=bv_s, in_=bv_d)
            nc.sync.dma_start(out=pb_s, in_=pb_d)
            nc.sync.dma_start(out=ident_s, in_=ident_d)
            nc.sync.dma_start(out=rmask_s, in_=rmask_d)
            nc.sync.dma_start(out=valid_s, in_=valid_d)

            qT_s = qy.tile([128, 16, 512], fr)
            yT_s = qy.tile([128, 16, 512], fr)
            kTl_s = kvloc.tile([128, 4, 512], fr)
            vAl_s = kvloc.tile([128, 4, 8, 65], fr)

            k_loc = dram.tile([4, 128, 512], fr)
            v_loc = dram.tile([4, 128, 8, 65], fr)
            k_gat = dram.tile([4, 4, 128, 512], fr)
            v_gat = dram.tile([4, 4, 128, 8, 65], fr)

            def rope_into(dst, ps, bias_col, rp):
                # dst[128, 512] (fp32r) <- rope(ps + bias); the 32-row
                # rotate-half swap is done by SBUF->SBUF DMAs (DVE needs
                # equal base partitions for two SBUF operands).
                tb = rp.tile([128, 512], fe, tag="tb", name="tb")
                nc.vector.tensor_scalar_add(tb, ps, bias_col)
                t2 = rp.tile([128, 512], fe, tag="t2", name="t2")
                nc.vector.tensor_mul(t2, tb, sin_s)
                tcs = rp.tile([128, 512], fe, tag="tc", name="tcs")
                nc.vector.tensor_mul(tcs, tb, cos_s)
                t2s = rp.tile([128, 512], fe, tag="t2s", name="t2s")
                for b0 in (0, 64):
                    nc.gpsimd.dma_start(out=t2s[b0:b0 + 32, :], in_=t2[b0 + 32:b0 + 64, :])
                    nc.gpsimd.dma_start(out=t2s[b0 + 32:b0 + 64, :], in_=t2[b0:b0 + 32, :])
                nc.vector.tensor_add(dst, t2s, tcs)

            # ---------------- phase 0: QKV projections ----------------
            with ExitStack() as p0:
                xp = p0.enter_context(tc.tile_pool(name="xp", bufs=1))
                rp = p0.enter_context(tc.tile_pool(name="rp", bufs=2))
                ps0 = p0.enter_context(tc.tile_pool(name="ps0", bufs=2, space="PSUM"))

                xT_s = xp.tile([128, 16, 512], fr)
                nc.sync.dma_start(out=xT_s, in_=xT_d.rearrange("(k p) t -> p k t", p=128))
                wq_all = xp.tile([128, 16, 2560], fr)
                for k4 in range(4):
                    nc.sync.dma_start(out=wq_all[:, k4 * 4:(k4 + 1) * 4, :],
                                      in_=wqkT_d[k4 * 512:(k4 + 1) * 512, :].rearrange("(j p) m -> p j m", p=128))
                wv_all = xp.tile([128, 16, 512], fr)
                nc.sync.dma_start(out=wv_all, in_=wvT_d.rearrange("(j p) m -> p j m", p=128))

                def qk_tile(dst, col0, bias_col):
                    ps = ps0.tile([128, 512], f32, tag="pk", name="ps")
                    for kc in range(16):
                        nc.tensor.matmul(ps, wq_all[:, kc, col0:col0 + 128], xT_s[:, kc, :],
                                         start=(kc == 0), stop=(kc == 15))
                    rope_into(dst, ps, bias_col, rp)

                # K^T tiles
                for kt in range(4):
                    qk_tile(kTl_s[:, kt, :], 2048 + kt * 128, bqk_s[:, 16 + kt:17 + kt])
                nc.sync.dma_start(out=k_loc.rearrange("k p t -> p k t"), in_=kTl_s)
                nc.gpsimd.collective_compute(
                    "AllGather", mybir.AluOpType.bypass, replica_groups=GROUPS,
                    ins=[k_loc.opt()], outs=[k_gat.opt()])

                # V tiles (natural layout, bias, ones column)
                ones_done = False
                psv = [ps0.tile([128, 512], f32, tag="pv", bufs=4, name=f"psv{mt}") for mt in range(4)]
                for kc in range(16):
                    for mt in range(4):
                        nc.tensor.matmul(psv[mt], xT_s[:, kc, mt * 128:(mt + 1) * 128], wv_all[:, kc, :],
                                         start=(kc == 0), stop=(kc == 15))
                nc.sync.dma_start(out=vAl_s[:, :, :, 64:65],
                                  in_=vones_d.rearrange("p c (g o) -> p c g o", o=1))
                for mt in range(4):
                    nc.vector.tensor_add(
                        vAl_s[:, mt, :, 0:64],
                        psv[mt].rearrange("p (g d) -> p g d", g=8),
                        bv_s.rearrange("p (g d) -> p g d", g=8))
                nc.sync.dma_start(out=v_loc.rearrange("c p g d -> p c g d"), in_=vAl_s)
                nc.gpsimd.collective_compute(
                    "AllGather", mybir.AluOpType.bypass, replica_groups=GROUPS,
                    ins=[v_loc.opt()], outs=[v_gat.opt()])

                # Q^T tiles
                for qt in range(16):
                    qk_tile(qT_s[:, qt, :], qt * 128, bqk_s[:, qt:qt + 1])

            # ---------------- phase 1: attention ----------------
            with ExitStack() as pa:
                at = pa.enter_context(tc.tile_pool(name="at", bufs=1))
                ep = pa.enter_context(tc.tile_pool(name="ep", bufs=6))
                psA = pa.enter_context(tc.tile_pool(name="psA", bufs=1, space="PSUM"))

                kg_s = at.tile([128, NPREF, 4, 128], fr)
                vg_s = at.tile([128, NPREF, 8, 65], fr)
                for s in range(NPREF):
                    rk, cp, half = slot_src(s)
                    nc.sync.dma_start(out=kg_s[:, s, :, :],
                                      in_=k_gat[rk, :, :, cp * 256 + half * 128: cp * 256 + (half + 1) * 128]
                                      .rearrange("k p t -> p k t"))
                    nc.sync.dma_start(out=vg_s[:, s, :, :], in_=v_gat[rk, cp * 2 + half, :, :, :])
                # stage qcB validity in place
                for s in range(NPREF):
                    nc.vector.tensor_scalar_mul(vg_s[:, s, :, :], vg_s[:, s, :, :].bitcast(fe), valid_s[:, s:s + 1])

                # Flat pipelined schedule over steps (qc, kt, slot, pair).
                # One step = one pair-position: 2 score matmuls (gi halves in
                # disjoint PE row groups), one gi-merged exp, 2 AV matmuls.
                # Lookahead-1 software pipeline crosses kt/qc boundaries so
                # neither the PE nor the scalar engine drains; avs PSUM
                # accumulators are copied to SBUF right at kt end so the next
                # kt's AV chain only waits on the copy, with the softmax
                # normalize chain running off the critical path.
                steps = []
                for qc, nslots in ((1, NSLOT_B), (0, NSLOT_A)):
                    for kt in range(4):
                        for slot in range(nslots):
                            for pair in range(2):
                                steps.append((qc, nslots, kt, slot, pair))

                avs_cur = {}

                def kv_slices(qc, kt, slot):
                    Ks, Vs = [], []
                    for gi in range(2):
                        g = 2 * kt + gi
                        if slot < 2:
                            Ks.append(kTl_s[gi * 64:(gi + 1) * 64, kt, qc * 256 + slot * 128: qc * 256 + (slot + 1) * 128])
                            Vs.append(vAl_s[:, qc * 2 + slot, g, :])
                        else:
                            Ks.append(kg_s[gi * 64:(gi + 1) * 64, slot - 2, kt, :])
                            Vs.append(vg_s[:, slot - 2, g, :])
                    return Ks, Vs

                def scores(step):
                    qc, nslots, kt, slot, pair = step
                    Ks, _ = kv_slices(qc, kt, slot)
                    pss = psA.tile([128, 1024], f32, tag="s", bufs=2, name="ps_s")
                    tA = kt * 4 + 2 * pair
                    masked = slot < 2
                    for gi in range(2):
                        nc.tensor.matmul(pss[:, gi * 512:(gi + 1) * 512], Ks[gi],
                                         qT_s[gi * 64:(gi + 1) * 64, tA:tA + 2, qc * 256:(qc + 1) * 256],
                                         start=True, stop=not masked)
                    if masked:
                        # additive causal mask via identity matmul (-240 on
                        # future positions): keeps the DVE out of the loop
                        for gi in range(2):
                            nc.tensor.matmul(pss[:, gi * 512:(gi + 1) * 512], ident_s,
                                             rmask_s[:, slot, :], start=False, stop=True)
                    return pss

                def expstep(step, pss):
                    e = ep.tile([128, 1024], fr, tag="e", bufs=12, name="e")
                    nc.scalar.activation(e, pss, AF.Exp, scale=0.125)
                    return e

                def av(step, e):
                    qc, nslots, kt, slot, pair = step
                    if slot == 0 and pair == 0:
                        avs_cur[(qc, kt)] = [[psA.tile([65, 512], f32, tag=f"av{gi}{j}", bufs=1,
                                                       name=f"ps_av{gi}{j}") for j in range(2)]
                                             for gi in range(2)]
                    avs = avs_cur[(qc, kt)]
                    _, Vs = kv_slices(qc, kt, slot)
                    for gi in range(2):
                        nc.tensor.matmul(avs[gi][pair], Vs[gi], e[:, gi * 512:(gi + 1) * 512],
                                         start=(slot == 0), stop=(slot == nslots - 1))

                def epilogue(qc, kt):
                    avs = avs_cur[(qc, kt)]
                    # One PSUM-freeing copy per accumulator, all emitted first:
                    # the next kt's AV chain only waits on these. The
                    # normalize chain then runs entirely off the SBUF copies.
                    yas = []
                    for gi in range(2):
                        for pair in range(2):
                            ya = ep.tile([65, 512], f32, tag="ya", bufs=4, name="ya")
                            nc.vector.tensor_copy(ya, avs[gi][pair])
                            yas.append(ya)
                    for gi in range(2):
                        for pair in range(2):
                            ya = yas[gi * 2 + pair]
                            rd = ep.tile([1, 512], f32, tag="rd", bufs=4, name="rd")
                            nc.vector.tensor_copy(rd, ya[64:65, :])
                            r_ = ep.tile([1, 512], f32, tag="r", bufs=4, name="r_")
                            nc.vector.reciprocal_approx_fast(r_, rd)
                            rb = ep.tile([64, 512], f32, tag="rb", bufs=4, name="rb")
                            nc.gpsimd.partition_broadcast(rb, r_)
                            tA = kt * 4 + 2 * pair
                            nc.vector.tensor_mul(
                                yT_s[gi * 64:(gi + 1) * 64, tA:tA + 2, qc * 256:(qc + 1) * 256],
                                ya[0:64, :].rearrange("p (j t) -> p j t", j=2),
                                rb.rearrange("p (j t) -> p j t", j=2))

                es_cur = expstep(steps[0], scores(steps[0]))
                for i, st in enumerate(steps):
                    nxt = steps[i + 1] if i + 1 < len(steps) else None
                    pss_nxt = scores(nxt) if nxt is not None else None
                    av(st, es_cur)
                    if nxt is not None:
                        es_cur = expstep(nxt, pss_nxt)
                    qc, nslots, kt, slot, pair = st
                    if slot == nslots - 1 and pair == 1:
                        epilogue(qc, kt)
                        if qc == 1 and kt == 3:
                            # tighten validity for qcA (valid_A subset of valid_B)
                            for s in range(6):
                                nc.vector.tensor_scalar_mul(vg_s[:, s, :, :], vg_s[:, s, :, :].bitcast(fe),
                                                            valid_s[:, 16 + s:17 + s])

            # ---------------- phase 2: output projection ----------------
            with ExitStack() as pp:
                pr = pp.enter_context(tc.tile_pool(name="pr", bufs=4))
                psP = pp.enter_context(tc.tile_pool(name="psP", bufs=8, space="PSUM"))
                for n in range(4):
                    pss = [psP.tile([128, 512], f32, tag="pp", name=f"pss{mt}") for mt in range(4)]
                    for kd in range(16):
                        wpj = pr.tile([128, 512], fr, tag="wpj", name="wpj")
                        nc.scalar.dma_start(out=wpj, in_=pwT_d[kd * 128:(kd + 1) * 128, n * 512:(n + 1) * 512])
                        for mt in range(4):
                            nc.tensor.matmul(pss[mt], yT_s[:, kd, mt * 128:(mt + 1) * 128], wpj,
                                             start=(kd == 0), stop=(kd == 15))
                    for mt in range(4):
                        ost = pr.tile([128, 512], f32, tag="ost", name="ost")
                        nc.vector.tensor_add(ost, pss[mt], pb_s[:, n * 512:(n + 1) * 512])
                        nc.sync.dma_start(out=out_d[mt * 128:(mt + 1) * 128, n * 512:(n + 1) * 512], in_=ost)

    nc.compile()
    _PROG["nc"] = nc
    return _PROG


def make_in_maps(x, cos, sin, attn_w, attn_b, proj_w, proj_b):
    x = np.asarray(x, np.float32)
    cos = np.asarray(cos, np.float32)
    sin = np.asarray(sin, np.float32)
    attn_w = np.asarray(attn_w, np.float32)
    attn_b = np.asarray(attn_b, np.float32)
    proj_w = np.asarray(proj_w, np.float32)
    proj_b = np.asarray(proj_b, np.float32)

    pq, pk, pv, py = perm_q(), perm_k(), perm_v(), perm_y()
    pqk = np.concatenate([pq, pk])
    wqkT = np.ascontiguousarray(attn_w[pqk, :].T)          # [2048, 2560]
    wvT = np.ascontiguousarray(attn_w[pv, :].T)            # [2048, 512]
    pwT = np.ascontiguousarray(proj_w.T[py, :])            # [2048, 2048]
    bqk = np.ascontiguousarray(attn_b[pqk].reshape(20, 128).T)   # [128, 20]
    bv = np.tile(attn_b[pv][None, :], (128, 1)).astype(np.float32)
    pb = np.tile(proj_b[None, :], (128, 1)).astype(np.float32)
    m0, m1 = host_masks()
    rmask = np.stack([m0, m1], axis=0).transpose(1, 0, 2)  # [128, 2, 256]
    rmask = np.concatenate([rmask] * 2, axis=2).copy()  # [128, 2, 512] (two q-tiles per pair-position)
    ident = np.eye(128, dtype=np.float32)

    if MM_DTYPE == "bf16":
        import ml_dtypes
        mmt = ml_dtypes.bfloat16
    else:
        mmt = np.float32
    wqkT = wqkT.astype(mmt)
    wvT = wvT.astype(mmt)
    pwT = pwT.astype(mmt)
    rmask = rmask.astype(mmt)
    ident = ident.astype(mmt)

    in_maps = []
    for c in range(NCORES):
        b, r = c // 4, c % 4
        ids = tok_ids(r)
        xT = np.ascontiguousarray(x[b, ids, :].T).astype(mmt)   # [2048, 512]
        cl = cos[ids, :].T                                  # [64, 512]
        sl = sin[ids, :].T.copy()
        sl[32:] *= -1.0
        cosT2 = np.concatenate([cl, cl], axis=0).astype(mmt)
        sinT2s = np.concatenate([sl, sl], axis=0).astype(mmt)
        vA, vB = valid_tables(r)
        valid = np.zeros((128, 32), np.float32)
        valid[:, 0:16] = vB[None, :]
        valid[:, 16:32] = vA[None, :]
        in_maps.append({
            "xT": xT, "wqkT": wqkT, "wvT": wvT, "pwT": pwT,
            "bqk": bqk, "bv": bv, "pb": pb,
            "cosT2": cosT2, "sinT2s": sinT2s, "ident": ident, "rmask": rmask, "valid": valid,
            "vones": np.ones((128, 4, 8), mmt),
        })
    return in_maps


def assemble_output(results):
    out = np.zeros((B, T, C), np.float32)
    for c in range(NCORES):
        b, r = c // 4, c % 4
        ids = tok_ids(r)
        out[b, ids, :] = results[c]["out"]
    return out


def kernel(**inputs):
    from concourse.bass_utils import run_bass_kernel_spmd

    prog = _build_program()
    in_maps = make_in_maps(**inputs)
    res = run_bass_kernel_spmd(prog["nc"], in_maps, list(range(NCORES)))
    return assemble_output(res.results)


if __name__ == "__main__":
    import reference

    inputs = {k: np.asarray(v) for k, v in reference.setup_inputs().items()}
    expected = np.asarray(reference.reference(**inputs))
    actual = kernel(**inputs)
    err = np.abs(actual - expected).max()
    rel = np.abs(actual - expected).max() / np.abs(expected).max()
    print(f"abs={err:.3e} rel={rel:.3e}")

